# revision 25
# baseline (speedup 1.0000x reference)
"""Contrastive-loss Trainium2 kernel: circulant-band symmetric decomposition.

sim = e@e.T is symmetric, so each unordered pair is computed once: every
row computes only the 4096 columns circularly AHEAD of it (distance
1..4096 mod 8192; distance exactly 4096 only counted from the first-half
row). Rows are label-sorted and rolled per core as before, so each core's
[1024, 4224] band is contiguous in its local column space. Head (first
128) and tail (last 128) band blocks get strict-triangular edge masks.

Each pair (i,j) contributes exp to row i (free-axis sums: ACT accum_out /
DVE accum) AND to row j (partition-axis sums): per-tile bf16 exp bands are
shift-accumulated into a column accumulator on DVE, then one all-ones
matmul reduces partitions at the end. Positives (same label) only occur
within 256 ahead, so the numerator needs just the leading 384 band
columns. Final loss assembly (log, valid gating, mean) happens on host
from per-core row/col partials.

Matmuls run in fp8 e4m3 DoubleRow (power-of-two quantization scale folded
into the Exp activation scale); exp is stored bf16, row-sums kept fp32.
"""

import contextlib
import ctypes
import os
import sys
import types

import ml_dtypes
import numpy as np

import concourse.bass as bass
import concourse.mybir as mybir
import concourse.tile as tile
from concourse.bass_utils import run_bass_kernel_spmd

# problem constants (hardcoded per task contract)
N, D, NCLS = 8192, 512, 512
TEMP = 0.07
EPS = 1e-8
M = 8            # cores
R = N // M       # 1024 rows per core
NT = R // 128    # 8 row-tiles per core
PAD = 256        # roll margin; must exceed max class size
W = 4224         # band width: 128 head + 3968 middle + 128 tail
HALF = 4096      # circular half-distance
CSPAN = (NT - 1) * 128 + W   # 5120: col span touched by one core's bands
NSPAN = (NT - 1) * 128 + 384  # 1280: numerator col span
CH = 512         # qT chunk width
KT = D // 128    # 4 contraction tiles

_AXON_SO = "/opt/axon/libaxon_pjrt.so"

LAST_RESULTS = None   # BassKernelResults of the most recent run (for test.py)


def _install_axon_trace_hook():
    """Provide antenv.axon_hooks (NTFF profiling) if the image lacks it."""
    try:
        from antenv.axon_hooks import get_axon_ntff_profile_hook  # noqa: F401
        return
    except ImportError:
        pass
    if not os.path.exists(_AXON_SO):
        return
    try:
        lib = ctypes.CDLL(_AXON_SO)
    except OSError:
        return
    if not hasattr(lib, "axon_start_nrt_profile"):
        return
    lib.axon_start_nrt_profile.argtypes = [ctypes.POINTER(ctypes.c_int64), ctypes.c_size_t]
    lib.axon_start_nrt_profile.restype = ctypes.c_int64
    lib.axon_stop_nrt_profile.argtypes = [ctypes.c_char_p]
    lib.axon_stop_nrt_profile.restype = ctypes.c_int64

    @contextlib.contextmanager
    def _hook(output_dir, device_ids):
        import jax
        jax.devices()
        if device_ids:
            ids = (ctypes.c_int64 * len(device_ids))(*device_ids)
            rc = lib.axon_start_nrt_profile(ids, len(device_ids))
        else:
            rc = lib.axon_start_nrt_profile(None, 0)
        if rc != 0:
            raise RuntimeError(f"axon_start_nrt_profile rc={rc}")
        try:
            yield
        finally:
            n = lib.axon_stop_nrt_profile(str(output_dir).encode())
            if n < 0:
                raise RuntimeError(f"axon_stop_nrt_profile rc={n}")

    _the_hook = [_hook]
    mod = types.ModuleType("antenv.axon_hooks")
    mod.set_axon_ntff_profile_hook = lambda h: _the_hook.__setitem__(0, h)
    mod.get_axon_ntff_profile_hook = lambda: _the_hook[0]
    sys.modules["antenv.axon_hooks"] = mod
    import antenv
    antenv.axon_hooks = mod


def _split_excess_waits(nc, max_waits=1):
    """This walrus build allows one sync-wait per instruction; move extras
    onto same-engine NoOps inserted just before (execution order preserved)."""
    for f in nc.m.functions:
        for b in f.blocks:
            insts = b.instructions
            new = []
            changed = False
            for inst in insts:
                si = inst.sync_info
                ow = list(si.on_wait) if (si and si.on_wait) else []
                if len(ow) > max_waits:
                    extra, keep = ow[:-max_waits], ow[-max_waits:]
                    for k, w in enumerate(extra):
                        nop = mybir.InstNoOp(name=f"{inst.name}-w{k}", ins=[], outs=[])
                        nop.engine = inst.engine
                        nop.sync_info = mybir.SyncInfo(on_wait=[w], on_update=[])
                        new.append(nop)
                    inst.sync_info = mybir.SyncInfo(
                        on_wait=keep,
                        on_update=list(si.on_update) if si.on_update else [])
                    changed = True
                new.append(inst)
            if changed:
                b.instructions = new


def _build_nc(k_exp):
    f32 = mybir.dt.float32
    bf16 = mybir.dt.bfloat16
    fp8 = mybir.dt.float8e4
    Alu = mybir.AluOpType
    Act = mybir.ActivationFunctionType
    DR = mybir.MatmulPerfMode.DoubleRow

    nc = bass.Bass(trn_type="TRN2", target_bir_lowering=False, debug=False)
    qT = nc.dram_tensor("qT", [128, (N // CH) * KT * CH], fp8, kind="ExternalInput")
    labd = nc.dram_tensor("lab", [N, 1], f32, kind="ExternalInput")
    triud = nc.dram_tensor("triu", [128, 128], bf16, kind="ExternalInput")
    taild = nc.dram_tensor("tailm", [128, 128], bf16, kind="ExternalInput")
    outd = nc.dram_tensor("out", [128, 64], f32, kind="ExternalOutput")
    coldend = nc.dram_tensor("colden", [128, CSPAN], bf16, kind="ExternalOutput")
    colnumd = nc.dram_tensor("colnum", [128, NSPAN], bf16, kind="ExternalOutput")

    # exp(sim/T) where psum holds sim * 4**k_exp (quantization scale folded in)
    act_scale = 1.0 / (TEMP * float(4.0 ** k_exp))

    with tile.TileContext(nc) as tc, contextlib.ExitStack() as ctx:
        qp = ctx.enter_context(tc.tile_pool(name="qp", bufs=1))
        pp = ctx.enter_context(tc.tile_pool(name="pp", bufs=2, space="PSUM"))
        ph = ctx.enter_context(tc.tile_pool(name="ph", bufs=2, space="PSUM"))
        wp = ctx.enter_context(tc.tile_pool(name="wp", bufs=2))
        sp = ctx.enter_context(tc.tile_pool(name="sp", bufs=1))

        # ---- preload: small tensors first ----
        labw = sp.tile([128, NSPAN + 384], f32)   # local col labels [0, 1664)
        nc.sync.dma_start(
            out=labw,
            in_=bass.AP(tensor=labd, offset=0, ap=[[0, 128], [1, NSPAN + 384]]))
        lab_rows = sp.tile([128, NT, 1], f32)
        nc.sync.dma_start(
            out=lab_rows,
            in_=labd[PAD:PAD + R, :].rearrange("(t p) o -> p t o", p=128))
        triu = sp.tile([128, 128], bf16)
        nc.sync.dma_start(out=triu, in_=triud.ap())
        tailm = sp.tile([128, 128], bf16)
        nc.sync.dma_start(out=tailm, in_=taild.ap())
        # bands only touch local cols [256, 5376) -> chunks 0..10; skip the rest
        qt = []
        for n in range((PAD + (NT - 1) * 128 + W + CH - 1) // CH):
            q = qp.tile([128, KT, CH], fp8, tag=f"q{n}")
            nc.sync.dma_start(
                out=q, in_=qT[:, n * KT * CH:(n + 1) * KT * CH])
            qt.append(q)

        band = sp.tile([128, NT, W], bf16)
        colaccB = sp.tile([128, CSPAN], bf16)
        nc.vector.memset(colaccB, 0.0)
        colnumB = sp.tile([128, NSPAN], bf16)
        nc.vector.memset(colnumB, 0.0)
        acc = sp.tile([128, 64], f32)
        nc.vector.memset(acc, 0.0)
        warm = sp.tile([128, 128], bf16)
        nc.vector.memset(warm, 0.0)
        warm_ps = pp.tile([128, 1024], f32, tag="ps")
        for w in range(48):
            nc.tensor.matmul(warm_ps[:, :128], warm, warm, start=True, stop=True)

        # ---- main loop: one 4224-wide band per 128-row tile ----
        for t in range(NT):
            base = PAD + t * 128          # abs local col of band start / row base
            a = base // CH                # lhsT chunk index
            off = base % CH               # lhsT offset within chunk
            lhs = qt[a]

            def mm(ps_slice, a0, wsub):
                # DR accumulation pair for one chunk-contained sub-range
                n0, co = a0 // CH, a0 % CH
                for k in range(0, KT, 2):
                    nc.tensor.matmul(
                        ps_slice, lhs[:, k:k + 2, off:off + 128],
                        qt[n0][:, k:k + 2, co:co + wsub],
                        start=(k == 0), stop=(k == KT - 2), perf_mode=DR)

            # head block [128,128] -> its own psum
            psh = ph.tile([128, 128], f32, tag="psh")
            mm(psh[:], base, 128)
            nc.scalar.activation(
                out=band[:, t, 0:128], in_=psh[:], func=Act.Exp, scale=act_scale)

            # middle [base+128, base+4096) = prefix p1 + 7 full chunks +
            # suffix s1; every matmul sub-range is contained in one 512 chunk
            # and each psum tile holds at most two accumulation subgroups
            p1 = (CH - (base + 128) % CH) % CH
            s1 = 384 - p1
            A1 = base + 128 + p1          # 512-aligned
            # group A: prefix + chunk 1
            ps = pp.tile([128, 1024], f32, tag="ps")
            if p1:
                mm(ps[:, 0:p1], base + 128, p1)
            mm(ps[:, p1:p1 + CH], A1, CH)
            nc.scalar.activation(
                out=band[:, t, 128:128 + p1 + CH], in_=ps[:, 0:p1 + CH],
                func=Act.Exp, scale=act_scale,
                accum_out=acc[:, t * 5:t * 5 + 1])
            # groups B, C, D: two full chunks each
            for gi in range(1, 4):
                A2 = A1 + CH + (gi - 1) * 1024
                boff = (A2 - base)
                ps = pp.tile([128, 1024], f32, tag="ps")
                mm(ps[:, 0:CH], A2, CH)
                mm(ps[:, CH:1024], A2 + CH, CH)
                nc.scalar.activation(
                    out=band[:, t, boff:boff + 1024], in_=ps[:], func=Act.Exp,
                    scale=act_scale, accum_out=acc[:, t * 5 + gi:t * 5 + gi + 1])
            # group E: suffix (middle, accum) + tail block (edge-masked later)
            pse = ph.tile([128, 512], f32, tag="pse")
            if s1:
                mm(pse[:, 0:s1], base + HALF - s1, s1)
                nc.scalar.activation(
                    out=band[:, t, HALF - s1:HALF], in_=pse[:, 0:s1],
                    func=Act.Exp, scale=act_scale,
                    accum_out=acc[:, t * 5 + 4:t * 5 + 5])
            mm(pse[:, s1:s1 + 128], base + HALF, 128)
            nc.scalar.activation(
                out=band[:, t, HALF:W], in_=pse[:, s1:s1 + 128],
                func=Act.Exp, scale=act_scale)

            # edge masks in place; masked row-sums via accum
            nc.vector.scalar_tensor_tensor(
                out=band[:, t, 0:128], in0=band[:, t, 0:128], scalar=1.0,
                in1=triu, op0=Alu.mult, op1=Alu.mult,
                accum_out=acc[:, 40 + t:41 + t])
            nc.vector.scalar_tensor_tensor(
                out=band[:, t, HALF:W], in0=band[:, t, HALF:W], scalar=1.0,
                in1=tailm, op0=Alu.mult, op1=Alu.mult,
                accum_out=acc[:, 48 + t:49 + t])

            # numerator: same-label & exp>1 gate on leading 384 band cols
            u = wp.tile([128, 384], bf16, tag="u")
            nc.vector.scalar_tensor_tensor(
                out=u, in0=labw[:, base:base + 384], scalar=lab_rows[:, t, :],
                in1=band[:, t, 0:384], op0=Alu.is_equal, op1=Alu.mult)
            u2 = wp.tile([128, 384], bf16, tag="u2")
            nc.vector.scalar_tensor_tensor(
                out=u2, in0=u, scalar=1.0, in1=u, op0=Alu.is_gt, op1=Alu.mult,
                accum_out=acc[:, 56 + t:57 + t])

            # shift-accumulate column partials (bf16, DVE 2x)
            nc.vector.tensor_tensor(
                out=colaccB[:, t * 128:t * 128 + W],
                in0=colaccB[:, t * 128:t * 128 + W],
                in1=band[:, t, :], op=Alu.add)
            nc.vector.tensor_tensor(
                out=colnumB[:, t * 128:t * 128 + 384],
                in0=colnumB[:, t * 128:t * 128 + 384],
                in1=u2, op=Alu.add)

        # ---- ship column partials; host reduces the 128 partitions in f64 ----
        nc.sync.dma_start(out=coldend.ap(), in_=colaccB)
        nc.sync.dma_start(out=colnumd.ap(), in_=colnumB)
        nc.sync.dma_start(out=outd.ap(), in_=acc)

    _split_excess_waits(nc)
    return nc


_NC_CACHE = {}


def _get_nc(k_exp):
    if k_exp not in _NC_CACHE:
        _NC_CACHE[k_exp] = _build_nc(k_exp)
    return _NC_CACHE[k_exp]


def _host_reference(emb, lab):
    """Numpy fallback (only for pathological label distributions where a
    class exceeds the PAD margin; never triggers for the target regime)."""
    e = emb / np.linalg.norm(emb, axis=1, keepdims=True).astype(np.float32)
    sim = (e @ e.T).astype(np.float32) / np.float32(TEMP)
    E = np.exp(sim, dtype=np.float32)
    pos = (lab[:, None] == lab[None, :]) & ~np.eye(len(lab), dtype=bool)
    valid = pos & (sim > 0)
    num = np.where(valid, E, 0).sum(1, dtype=np.float32)
    den = E.sum(1, dtype=np.float32) - np.diagonal(E)
    rv = valid.any(1) & (den > 0)
    ns = np.where(rv, num, np.float32(1.0))
    ds = np.where(rv, den, np.float32(1.0))
    li = np.log(ds + np.float32(EPS)) - np.log(ns)
    nv = int(rv.sum())
    if nv == 0:
        return np.float32(0.0)
    return np.float32(abs(float(np.where(rv, li, 0).sum(dtype=np.float64)) / nv))


def kernel(**inputs):
    global LAST_RESULTS
    emb = np.ascontiguousarray(np.asarray(inputs["embeddings"], dtype=np.float32))
    lab = np.asarray(inputs["labels"]).astype(np.int64).ravel()
    assert emb.shape == (N, D) and lab.shape == (N,)

    if np.bincount(lab, minlength=1).max() > PAD:
        return _host_reference(emb, lab)

    _install_axon_trace_hook()

    # host prep: normalize, sort by label, per-core roll + transpose
    e = emb / np.linalg.norm(emb, axis=1, keepdims=True).astype(np.float32)
    order = np.argsort(lab, kind="stable")
    es = np.ascontiguousarray(e[order])
    ls = lab[order].astype(np.float32)

    # fp8 e4m3 quantization with a power-of-two scale (keeps relative
    # precision exactly scale-invariant; act_scale compile-time per k_exp)
    absmax = float(np.abs(es).max())
    k_exp = int(np.floor(np.log2(240.0 / max(absmax, 1e-30))))
    k_exp = max(min(k_exp, 14), -14)
    es = (es * np.float32(2.0 ** k_exp)).astype(ml_dtypes.float8_e4m3)

    ri = np.arange(128)
    triu = (ri[None, :] > ri[:, None]).astype(ml_dtypes.bfloat16)
    tail_incl = (ri[None, :] <= ri[:, None]).astype(ml_dtypes.bfloat16)
    tail_strict = (ri[None, :] < ri[:, None]).astype(ml_dtypes.bfloat16)

    in_maps = []
    for c in range(M):
        shift = c * R - PAD
        rolled = np.roll(es, -shift, axis=0)         # [N, D] fp8
        labr = np.roll(ls, -shift).reshape(N, 1)     # [N, 1] f32
        qTc = (rolled.T.reshape(KT, 128, N // CH, CH)
               .transpose(1, 2, 0, 3)
               .reshape(128, (N // CH) * KT * CH))
        qTc = np.ascontiguousarray(qTc)
        in_maps.append({
            "qT": qTc,
            "lab": np.ascontiguousarray(labr),
            "triu": triu,
            "tailm": tail_incl if c < 4 else tail_strict,
        })

    nc = _get_nc(k_exp)
    res = run_bass_kernel_spmd(nc, in_maps, core_ids=list(range(M)))
    LAST_RESULTS = res

    # ---- host assembly of row/col partials ----
    num = np.zeros(N, np.float64)
    den = np.zeros(N, np.float64)
    idx = np.arange(CSPAN)
    for c in range(M):
        o = res.results[c]["out"].astype(np.float64)       # [128, 64]
        den_rows = (o[:, :40].reshape(128, NT, 5).sum(2)
                    + o[:, 40:48] + o[:, 48:56])           # [p, t]
        num_rows = o[:, 56:64]
        g0 = c * R
        den[g0:g0 + R] += den_rows.T.ravel()
        num[g0:g0 + R] += num_rows.T.ravel()
        jj = (g0 + idx) % N
        np.add.at(den, jj,
                  res.results[c]["colden"].astype(np.float64).sum(0))
        np.add.at(num, jj[:NSPAN],
                  res.results[c]["colnum"].astype(np.float64).sum(0))

    global LAST_PARTIALS
    LAST_PARTIALS = (num.copy(), den.copy())
    valid = (num > 0) & (den > 0)
    nv = int(valid.sum())
    if nv == 0:
        return np.float32(0.0)
    ns = np.where(valid, num, 1.0)
    ds = np.where(valid, den, 1.0)
    li = np.log(ds + EPS) - np.log(ns)
    return np.float32(abs(float(li[valid].sum()) / nv))


# revision 26
# speedup vs baseline: 1.0062x; 1.0062x over previous
"""Contrastive-loss Trainium2 kernel: circulant-band symmetric decomposition.

sim = e@e.T is symmetric, so each unordered pair is computed once: every
row computes only the 4096 columns circularly AHEAD of it (distance
1..4096 mod 8192; distance exactly 4096 only counted from the first-half
row). Rows are label-sorted and rolled per core as before, so each core's
[1024, 4224] band is contiguous in its local column space. Head (first
128) and tail (last 128) band blocks get strict-triangular edge masks.

Each pair (i,j) contributes exp to row i (free-axis sums: ACT accum_out /
DVE accum) AND to row j (partition-axis sums): per-tile bf16 exp bands are
shift-accumulated into a column accumulator on DVE, then one all-ones
matmul reduces partitions at the end. Positives (same label) only occur
within 256 ahead, so the numerator needs just the leading 384 band
columns. Final loss assembly (log, valid gating, mean) happens on host
from per-core row/col partials.

Matmuls run in fp8 e4m3 DoubleRow (power-of-two quantization scale folded
into the Exp activation scale); exp is stored bf16, row-sums kept fp32.
"""

import contextlib
import ctypes
import os
import sys
import types

import ml_dtypes
import numpy as np

import concourse.bass as bass
import concourse.mybir as mybir
import concourse.tile as tile
from concourse.bass_utils import run_bass_kernel_spmd

# problem constants (hardcoded per task contract)
N, D, NCLS = 8192, 512, 512
TEMP = 0.07
EPS = 1e-8
M = 8            # cores
R = N // M       # 1024 rows per core
NT = R // 128    # 8 row-tiles per core
PAD = 256        # roll margin; must exceed max class size
W = 4224         # band width: 128 head + 3968 middle + 128 tail
HALF = 4096      # circular half-distance
CSPAN = (NT - 1) * 128 + W   # 5120: col span touched by one core's bands
NSPAN = (NT - 1) * 128 + 384  # 1280: numerator col span
CH = 512         # qT chunk width
KT = D // 128    # 4 contraction tiles

_AXON_SO = "/opt/axon/libaxon_pjrt.so"

LAST_RESULTS = None   # BassKernelResults of the most recent run (for test.py)


def _install_axon_trace_hook():
    """Provide antenv.axon_hooks (NTFF profiling) if the image lacks it."""
    try:
        from antenv.axon_hooks import get_axon_ntff_profile_hook  # noqa: F401
        return
    except ImportError:
        pass
    if not os.path.exists(_AXON_SO):
        return
    try:
        lib = ctypes.CDLL(_AXON_SO)
    except OSError:
        return
    if not hasattr(lib, "axon_start_nrt_profile"):
        return
    lib.axon_start_nrt_profile.argtypes = [ctypes.POINTER(ctypes.c_int64), ctypes.c_size_t]
    lib.axon_start_nrt_profile.restype = ctypes.c_int64
    lib.axon_stop_nrt_profile.argtypes = [ctypes.c_char_p]
    lib.axon_stop_nrt_profile.restype = ctypes.c_int64

    @contextlib.contextmanager
    def _hook(output_dir, device_ids):
        import jax
        jax.devices()
        if device_ids:
            ids = (ctypes.c_int64 * len(device_ids))(*device_ids)
            rc = lib.axon_start_nrt_profile(ids, len(device_ids))
        else:
            rc = lib.axon_start_nrt_profile(None, 0)
        if rc != 0:
            raise RuntimeError(f"axon_start_nrt_profile rc={rc}")
        try:
            yield
        finally:
            n = lib.axon_stop_nrt_profile(str(output_dir).encode())
            if n < 0:
                raise RuntimeError(f"axon_stop_nrt_profile rc={n}")

    _the_hook = [_hook]
    mod = types.ModuleType("antenv.axon_hooks")
    mod.set_axon_ntff_profile_hook = lambda h: _the_hook.__setitem__(0, h)
    mod.get_axon_ntff_profile_hook = lambda: _the_hook[0]
    sys.modules["antenv.axon_hooks"] = mod
    import antenv
    antenv.axon_hooks = mod


def _split_excess_waits(nc, max_waits=1):
    """This walrus build allows one sync-wait per instruction; move extras
    onto same-engine NoOps inserted just before (execution order preserved)."""
    for f in nc.m.functions:
        for b in f.blocks:
            insts = b.instructions
            new = []
            changed = False
            for inst in insts:
                si = inst.sync_info
                ow = list(si.on_wait) if (si and si.on_wait) else []
                if len(ow) > max_waits:
                    extra, keep = ow[:-max_waits], ow[-max_waits:]
                    for k, w in enumerate(extra):
                        nop = mybir.InstNoOp(name=f"{inst.name}-w{k}", ins=[], outs=[])
                        nop.engine = inst.engine
                        nop.sync_info = mybir.SyncInfo(on_wait=[w], on_update=[])
                        new.append(nop)
                    inst.sync_info = mybir.SyncInfo(
                        on_wait=keep,
                        on_update=list(si.on_update) if si.on_update else [])
                    changed = True
                new.append(inst)
            if changed:
                b.instructions = new


def _build_nc(k_exp):
    f32 = mybir.dt.float32
    bf16 = mybir.dt.bfloat16
    fp8 = mybir.dt.float8e4
    Alu = mybir.AluOpType
    Act = mybir.ActivationFunctionType
    DR = mybir.MatmulPerfMode.DoubleRow

    nc = bass.Bass(trn_type="TRN2", target_bir_lowering=False, debug=False)
    qT = nc.dram_tensor("qT", [128, (N // CH) * KT * CH], fp8, kind="ExternalInput")
    labd = nc.dram_tensor("lab", [N, 1], f32, kind="ExternalInput")
    triud = nc.dram_tensor("triu", [128, 128], bf16, kind="ExternalInput")
    taild = nc.dram_tensor("tailm", [128, 128], bf16, kind="ExternalInput")
    outd = nc.dram_tensor("out", [128, 64], f32, kind="ExternalOutput")
    coldend = nc.dram_tensor("colden", [128, CSPAN], bf16, kind="ExternalOutput")
    colnumd = nc.dram_tensor("colnum", [128, NSPAN], bf16, kind="ExternalOutput")

    # exp(sim/T) where psum holds sim * 4**k_exp (quantization scale folded in)
    act_scale = 1.0 / (TEMP * float(4.0 ** k_exp))

    with tile.TileContext(nc) as tc, contextlib.ExitStack() as ctx:
        qp = ctx.enter_context(tc.tile_pool(name="qp", bufs=1))
        pp = ctx.enter_context(tc.tile_pool(name="pp", bufs=2, space="PSUM"))
        ph = ctx.enter_context(tc.tile_pool(name="ph", bufs=2, space="PSUM"))
        wp = ctx.enter_context(tc.tile_pool(name="wp", bufs=2))
        sp = ctx.enter_context(tc.tile_pool(name="sp", bufs=1))

        # ---- preload: small tensors first ----
        labw = sp.tile([128, NSPAN + 384], f32)   # local col labels [0, 1664)
        nc.sync.dma_start(
            out=labw,
            in_=bass.AP(tensor=labd, offset=0, ap=[[0, 128], [1, NSPAN + 384]]))
        lab_rows = sp.tile([128, NT, 1], f32)
        nc.sync.dma_start(
            out=lab_rows,
            in_=labd[PAD:PAD + R, :].rearrange("(t p) o -> p t o", p=128))
        triu = sp.tile([128, 128], bf16)
        nc.sync.dma_start(out=triu, in_=triud.ap())
        tailm = sp.tile([128, 128], bf16)
        nc.sync.dma_start(out=tailm, in_=taild.ap())
        qt = []
        for n in range(N // CH):
            q = qp.tile([128, KT, CH], fp8, tag=f"q{n}")
            nc.sync.dma_start(
                out=q, in_=qT[:, n * KT * CH:(n + 1) * KT * CH])
            qt.append(q)

        band = sp.tile([128, NT, W], bf16)
        colaccB = sp.tile([128, CSPAN], bf16)
        nc.vector.memset(colaccB, 0.0)
        colnumB = sp.tile([128, NSPAN], bf16)
        nc.vector.memset(colnumB, 0.0)
        acc = sp.tile([128, 64], f32)
        nc.vector.memset(acc, 0.0)
        warm = sp.tile([128, 128], bf16)
        nc.vector.memset(warm, 0.0)
        warm_ps = pp.tile([128, 1024], f32, tag="ps")
        for w in range(48):
            nc.tensor.matmul(warm_ps[:, :128], warm, warm, start=True, stop=True)

        # ---- main loop: one 4224-wide band per 128-row tile ----
        for t in range(NT):
            base = PAD + t * 128          # abs local col of band start / row base
            a = base // CH                # lhsT chunk index
            off = base % CH               # lhsT offset within chunk
            lhs = qt[a]

            def mm(ps_slice, a0, wsub):
                # DR accumulation pair for one chunk-contained sub-range
                n0, co = a0 // CH, a0 % CH
                for k in range(0, KT, 2):
                    nc.tensor.matmul(
                        ps_slice, lhs[:, k:k + 2, off:off + 128],
                        qt[n0][:, k:k + 2, co:co + wsub],
                        start=(k == 0), stop=(k == KT - 2), perf_mode=DR)

            # head block [128,128] -> its own psum
            psh = ph.tile([128, 128], f32, tag="psh")
            mm(psh[:], base, 128)
            nc.scalar.activation(
                out=band[:, t, 0:128], in_=psh[:], func=Act.Exp, scale=act_scale)

            # middle [base+128, base+4096) = prefix p1 + 7 full chunks +
            # suffix s1; every matmul sub-range is contained in one 512 chunk
            # and each psum tile holds at most two accumulation subgroups
            p1 = (CH - (base + 128) % CH) % CH
            s1 = 384 - p1
            A1 = base + 128 + p1          # 512-aligned
            # group A: prefix + chunk 1
            ps = pp.tile([128, 1024], f32, tag="ps")
            if p1:
                mm(ps[:, 0:p1], base + 128, p1)
            mm(ps[:, p1:p1 + CH], A1, CH)
            nc.scalar.activation(
                out=band[:, t, 128:128 + p1 + CH], in_=ps[:, 0:p1 + CH],
                func=Act.Exp, scale=act_scale,
                accum_out=acc[:, t * 5:t * 5 + 1])
            # groups B, C, D: two full chunks each
            for gi in range(1, 4):
                A2 = A1 + CH + (gi - 1) * 1024
                boff = (A2 - base)
                ps = pp.tile([128, 1024], f32, tag="ps")
                mm(ps[:, 0:CH], A2, CH)
                mm(ps[:, CH:1024], A2 + CH, CH)
                nc.scalar.activation(
                    out=band[:, t, boff:boff + 1024], in_=ps[:], func=Act.Exp,
                    scale=act_scale, accum_out=acc[:, t * 5 + gi:t * 5 + gi + 1])
            # group E: suffix (middle, accum) + tail block (edge-masked later)
            pse = ph.tile([128, 512], f32, tag="pse")
            if s1:
                mm(pse[:, 0:s1], base + HALF - s1, s1)
                nc.scalar.activation(
                    out=band[:, t, HALF - s1:HALF], in_=pse[:, 0:s1],
                    func=Act.Exp, scale=act_scale,
                    accum_out=acc[:, t * 5 + 4:t * 5 + 5])
            mm(pse[:, s1:s1 + 128], base + HALF, 128)
            nc.scalar.activation(
                out=band[:, t, HALF:W], in_=pse[:, s1:s1 + 128],
                func=Act.Exp, scale=act_scale)

            # edge masks in place; masked row-sums via accum
            nc.vector.scalar_tensor_tensor(
                out=band[:, t, 0:128], in0=band[:, t, 0:128], scalar=1.0,
                in1=triu, op0=Alu.mult, op1=Alu.mult,
                accum_out=acc[:, 40 + t:41 + t])
            nc.vector.scalar_tensor_tensor(
                out=band[:, t, HALF:W], in0=band[:, t, HALF:W], scalar=1.0,
                in1=tailm, op0=Alu.mult, op1=Alu.mult,
                accum_out=acc[:, 48 + t:49 + t])

            # numerator: same-label & exp>1 gate on leading 384 band cols
            u = wp.tile([128, 384], bf16, tag="u")
            nc.vector.scalar_tensor_tensor(
                out=u, in0=labw[:, base:base + 384], scalar=lab_rows[:, t, :],
                in1=band[:, t, 0:384], op0=Alu.is_equal, op1=Alu.mult)
            u2 = wp.tile([128, 384], bf16, tag="u2")
            nc.vector.scalar_tensor_tensor(
                out=u2, in0=u, scalar=1.0, in1=u, op0=Alu.is_gt, op1=Alu.mult,
                accum_out=acc[:, 56 + t:57 + t])

            # shift-accumulate column partials (bf16, DVE 2x)
            nc.vector.tensor_tensor(
                out=colaccB[:, t * 128:t * 128 + W],
                in0=colaccB[:, t * 128:t * 128 + W],
                in1=band[:, t, :], op=Alu.add)
            nc.vector.tensor_tensor(
                out=colnumB[:, t * 128:t * 128 + 384],
                in0=colnumB[:, t * 128:t * 128 + 384],
                in1=u2, op=Alu.add)

        # ---- ship column partials; host reduces the 128 partitions in f64 ----
        nc.sync.dma_start(out=coldend.ap(), in_=colaccB)
        nc.sync.dma_start(out=colnumd.ap(), in_=colnumB)
        nc.sync.dma_start(out=outd.ap(), in_=acc)

    _split_excess_waits(nc)
    return nc


_NC_CACHE = {}


def _get_nc(k_exp):
    if k_exp not in _NC_CACHE:
        _NC_CACHE[k_exp] = _build_nc(k_exp)
    return _NC_CACHE[k_exp]


def _host_reference(emb, lab):
    """Numpy fallback (only for pathological label distributions where a
    class exceeds the PAD margin; never triggers for the target regime)."""
    e = emb / np.linalg.norm(emb, axis=1, keepdims=True).astype(np.float32)
    sim = (e @ e.T).astype(np.float32) / np.float32(TEMP)
    E = np.exp(sim, dtype=np.float32)
    pos = (lab[:, None] == lab[None, :]) & ~np.eye(len(lab), dtype=bool)
    valid = pos & (sim > 0)
    num = np.where(valid, E, 0).sum(1, dtype=np.float32)
    den = E.sum(1, dtype=np.float32) - np.diagonal(E)
    rv = valid.any(1) & (den > 0)
    ns = np.where(rv, num, np.float32(1.0))
    ds = np.where(rv, den, np.float32(1.0))
    li = np.log(ds + np.float32(EPS)) - np.log(ns)
    nv = int(rv.sum())
    if nv == 0:
        return np.float32(0.0)
    return np.float32(abs(float(np.where(rv, li, 0).sum(dtype=np.float64)) / nv))


def kernel(**inputs):
    global LAST_RESULTS
    emb = np.ascontiguousarray(np.asarray(inputs["embeddings"], dtype=np.float32))
    lab = np.asarray(inputs["labels"]).astype(np.int64).ravel()
    assert emb.shape == (N, D) and lab.shape == (N,)

    if np.bincount(lab, minlength=1).max() > PAD:
        return _host_reference(emb, lab)

    _install_axon_trace_hook()

    # host prep: normalize, sort by label, per-core roll + transpose
    e = emb / np.linalg.norm(emb, axis=1, keepdims=True).astype(np.float32)
    order = np.argsort(lab, kind="stable")
    es = np.ascontiguousarray(e[order])
    ls = lab[order].astype(np.float32)

    # fp8 e4m3 quantization with a power-of-two scale (keeps relative
    # precision exactly scale-invariant; act_scale compile-time per k_exp)
    absmax = float(np.abs(es).max())
    k_exp = int(np.floor(np.log2(240.0 / max(absmax, 1e-30))))
    k_exp = max(min(k_exp, 14), -14)
    es = (es * np.float32(2.0 ** k_exp)).astype(ml_dtypes.float8_e4m3)

    ri = np.arange(128)
    triu = (ri[None, :] > ri[:, None]).astype(ml_dtypes.bfloat16)
    tail_incl = (ri[None, :] <= ri[:, None]).astype(ml_dtypes.bfloat16)
    tail_strict = (ri[None, :] < ri[:, None]).astype(ml_dtypes.bfloat16)

    in_maps = []
    for c in range(M):
        shift = c * R - PAD
        rolled = np.roll(es, -shift, axis=0)         # [N, D] fp8
        labr = np.roll(ls, -shift).reshape(N, 1)     # [N, 1] f32
        qTc = (rolled.T.reshape(KT, 128, N // CH, CH)
               .transpose(1, 2, 0, 3)
               .reshape(128, (N // CH) * KT * CH))
        qTc = np.ascontiguousarray(qTc)
        in_maps.append({
            "qT": qTc,
            "lab": np.ascontiguousarray(labr),
            "triu": triu,
            "tailm": tail_incl if c < 4 else tail_strict,
        })

    nc = _get_nc(k_exp)
    res = run_bass_kernel_spmd(nc, in_maps, core_ids=list(range(M)))
    LAST_RESULTS = res

    # ---- host assembly of row/col partials ----
    num = np.zeros(N, np.float64)
    den = np.zeros(N, np.float64)
    idx = np.arange(CSPAN)
    for c in range(M):
        o = res.results[c]["out"].astype(np.float64)       # [128, 64]
        den_rows = (o[:, :40].reshape(128, NT, 5).sum(2)
                    + o[:, 40:48] + o[:, 48:56])           # [p, t]
        num_rows = o[:, 56:64]
        g0 = c * R
        den[g0:g0 + R] += den_rows.T.ravel()
        num[g0:g0 + R] += num_rows.T.ravel()
        jj = (g0 + idx) % N
        np.add.at(den, jj,
                  res.results[c]["colden"].astype(np.float64).sum(0))
        np.add.at(num, jj[:NSPAN],
                  res.results[c]["colnum"].astype(np.float64).sum(0))

    global LAST_PARTIALS
    LAST_PARTIALS = (num.copy(), den.copy())
    valid = (num > 0) & (den > 0)
    nv = int(valid.sum())
    if nv == 0:
        return np.float32(0.0)
    ns = np.where(valid, num, 1.0)
    ds = np.where(valid, den, 1.0)
    li = np.log(ds + EPS) - np.log(ns)
    return np.float32(abs(float(li[valid].sum()) / nv))
